# revision 32
# baseline (speedup 1.0000x reference)
"""Trainium2 Bass kernel for nn_Attention_86199993631321.

Reference computation (B=8, N=128, H=512):
    pair[b,i,j,:] = x[b,i,:] + x[b,j,:]
    out = pair @ W.T + b                # [B, N, N, H]

Algebra: out[b,i,j,:] = P[b,i,:] + P[b,j,:] with P = x @ W.T + 0.5*b.
Sharding: data-parallel over batch (core b handles batch b).

v3 design notes (HW facts measured on this setup):
  - PE runs at a fixed 1.2 GHz (no HAM warm-up observed); at most ~2
    matmul streams overlap across row-groups.  FD<=512 per matmul (one
    f32 PSUM bank).
  - PSUM->SBUF eviction runs at 1x everywhere (fp32 source), ~1 elem/
    cycle/lane: ACT (1.2GHz) and DVE (0.96GHz) are the only PSUM
    readers; GPSIMD has no PSUM port.
  - DMA: full-128-partition sources stream at ~370 GB/s; partition
    subranges lose proportional bandwidth ([0,64) even SDMA engines,
    [64,128) odd).
  - Output is symmetric: only the block-lower-triangle (8-col blocks)
    is computed/written (8.7MB bf16/core); host mirrors + upcasts.
  - Column blocks pack into 128-partition tiles at 32-row granularity:
    blocks 0-3 full; (4,12),(5,13),(6,14),(7,15) as 96+32; (8,10),
    (9,11) as 64+64.  Each btile = 4 groups of [128, 2*512] f32 (2
    PSUM banks); psum pool bufs=4 decouples the pipeline.
  - Per group (quadrant q = its column pair): K=1 (full tiles) or K=2
    (packed pairs, host-built 0/1 partition masks select seg A/B rows)
    broadcast matmuls write P[j]; the i term P[sigma(p)] is added by
    route: TV (DVE TT from PSUM), SV (ACT copy + DVE bf16 TT), SG
    (ACT copy + GPSIMD TT), IA (PE sigma-permutation matmul
    accumulate + ACT copy).
"""

import sys

if "/opt/trn_rl_repo" not in sys.path:
    sys.path.insert(0, "/opt/trn_rl_repo")

import numpy as np

B, N, H = 8, 128, 512
NCORES = 8
KC = H // 128
WXW = N + H + 128  # x.T | W.T | ones col
# perm/mask input [128, 640] bf16:
#   0:128 identity, 128:256 sigma96, 256:384 sigma64,
#   384:512 pair masks type96 (row 32q: [p<96], row 32q+1: [p>=96]),
#   512:640 pair masks type64
# F/pair btiles interleaved to smooth per-engine load; ends on a
# full-width F block so the DMA drain tail runs at full rate
BTILES = [
    ("F", 0, None),
    ("P96", 4, 12),
    ("F", 1, None),
    ("P96", 5, 13),
    ("F", 2, None),
    ("P96", 6, 14),
    ("P64", 8, 10),
    ("P96", 7, 15),
    ("P64", 9, 11),
    ("F", 3, None),
]
# 40 groups (4 per btile, quadrant-ordered)
ROUTES = [
    "TV", "SV", "SG", "TV",   # F0
    "SG", "TV", "IA", "SV",   # P96 (4,12)
    "TV", "SG", "SV", "TV",   # F1
    "IA", "SV", "TV", "SG",   # P96 (5,13)
    "TV", "IA", "SG", "TV",   # F2
    "SG", "TV", "IA", "SV",   # P96 (6,14)
    "TV", "SV", "TV", "SG",   # P64 (8,10)
    "SG", "TV", "IA", "SV",   # P96 (7,15)
    "TV", "IA", "SV", "TV",   # P64 (9,11)
    "SG", "TV", "IA", "TV",   # F3
]

_BUILT = {}


def _build_nc():
    import concourse.bass as bass
    import concourse.bacc as bacc
    import concourse.tile as tile
    from concourse import mybir

    f32 = mybir.dt.float32
    bf16 = mybir.dt.bfloat16
    ADD = mybir.AluOpType.add
    COPY = mybir.ActivationFunctionType.Copy

    nc = bacc.Bacc()
    wx_ext = nc.declare_dram_parameter("wx", [H, WXW], bf16, isOutput=False)
    hb_ext = nc.declare_dram_parameter("halfb", [1, H], bf16, isOutput=False)
    perm_ext = nc.declare_dram_parameter("perm", [128, 640], bf16, isOutput=False)
    out_ext = nc.declare_dram_parameter("out", [N, N, H], bf16, isOutput=True)

    group_idx = [0]
    dma_idx = [0]

    with tile.TileContext(nc) as tc:
        with (
            tc.tile_pool(name="const", bufs=1) as const,
            tc.tile_pool(name="stage", bufs=10) as stage,
            tc.tile_pool(name="bcast", bufs=4) as bcast,
            tc.tile_pool(name="outp", bufs=4) as outp,
            tc.tile_pool(name="psum", bufs=4, space="PSUM") as psum,
            tc.tile_pool(name="dscr", bufs=1, space="DRAM") as dscr,
        ):
            # ---- load packed inputs ----
            wx_sb = const.tile([128, KC, WXW], bf16)
            wx_v = wx_ext.rearrange("(c p) m -> p c m", p=128)
            for c in range(KC):
                eng = nc.sync if c % 2 == 0 else nc.scalar
                eng.dma_start(out=wx_sb[:, c, :], in_=wx_v[:, c, :])
            perm_sb = const.tile([128, 640], bf16)
            nc.sync.dma_start(out=perm_sb, in_=perm_ext[:, :])
            ones_sb = const.tile([128, 128], bf16)
            nc.vector.memset(ones_sb, 1.0)
            hb_sb = const.tile([1, H], bf16)
            nc.gpsimd.dma_start(out=hb_sb, in_=hb_ext[:, :])

            # ---- P = x @ W.T + 0.5*b -> PSUM [128, 512] ----
            ps_proj = psum.tile([128, 2 * H], f32, tag="ps")
            for c in range(KC):
                nc.tensor.matmul(
                    ps_proj[:, 0:H],
                    wx_sb[:, c, 0:N],
                    wx_sb[:, c, N : N + H],
                    start=(c == 0),
                    stop=False,
                )
            nc.tensor.matmul(
                ps_proj[:, 0:H],
                wx_sb[0:1, 0, N + H : N + H + 128],
                hb_sb,
                start=False,
                stop=True,
            )

            # ---- P replicated 2x along free dim (bf16) ----
            P_rep = const.tile([128, 2, H], bf16)
            nc.scalar.activation(P_rep[:, 0, :], ps_proj[:, 0:H], COPY)
            nc.vector.tensor_copy(P_rep[:, 1, :], P_rep[:, 0, :])

            # Chunk staging via a DRAM round trip: one SBUF->DRAM write of
            # P, then a few affine gather DMAs build the per-quadrant
            # chunk layout C[32q + r, bi, s*H:] = P[8*k_{A/B}(bi)+2q+s].
            # This replaces 16 small SWDGE descriptor-generation DMAs.
            # F0's chunk is staged directly from SBUF (GPS) for the ramp.
            C = const.tile([128, 10, 2 * H], bf16)
            ch0 = const.tile([128, 2 * H], bf16)
            nc.gpsimd.dma_start(out=ch0[0:128:32, :], in_=P_rep[0:8, 0, :])

            P_dram = dscr.tile([128, H], bf16, name="pdram")
            nc.sync.dma_start(out=P_dram[:, :], in_=P_rep[:, 0, :])
            # C's slot dim is indexed by block k of the A-seg; B rows land
            # at the SAME slot (kB = kA+8 for P96, kA+2 for P64), so the
            # K=2 matmul reads A/B on adjacent partitions at one offset.
            # Rows j = 8k + 2q + s: view (q, k, (s h)) is a clean 3-dim AP.
            pview = P_dram.rearrange("(k q s) h -> q k (s h)", k=16, q=4, s=2)
            nc.sync.dma_start(out=C[0:128:32, 0:10, :], in_=pview[:, 0:10, :])
            nc.sync.dma_start(out=C[1:128:32, 4:8, :], in_=pview[:, 12:16, :])
            nc.sync.dma_start(out=C[1:128:32, 8:10, :], in_=pview[:, 10:12, :])

            # sigma-shifted copies for TT routes on packed tiles (GPS)
            P_g96 = const.tile([128, 2, H], bf16)
            P_g64 = const.tile([128, 2, H], bf16)
            nc.gpsimd.dma_start(out=P_g96[0:96, :, :], in_=P_rep[32:128, :, :])
            nc.gpsimd.dma_start(out=P_g96[96:128, :, :], in_=P_rep[96:128, :, :])
            nc.gpsimd.dma_start(out=P_g64[0:64, :, :], in_=P_rep[64:128, :, :])
            nc.gpsimd.dma_start(out=P_g64[64:128, :, :], in_=P_rep[64:128, :, :])
            SIG = {"F": P_rep, "P96": P_g96, "P64": P_g64}
            POFF = {"F": 0, "P96": 128, "P64": 256}
            MOFF = {"P96": 384, "P64": 512}

            def next_route():
                r = ROUTES[group_idx[0] % len(ROUTES)]
                group_idx[0] += 1
                return r

            def out_dma(dst, src, small=False):
                # large writes on the sync ring; small ones on the scalar
                # ring (cheap to issue) to keep the sync issue queue short
                dma_idx[0] += 1
                eng = nc.scalar if small else nc.sync
                eng.dma_start(out=dst, in_=src)

            def do_btile(bi, kind, kA, kB):
                # chunk: quadrant q partition 32q holds P rows (8kA+2q,
                # 8kA+2q+1); packed btiles add kB rows at partition 32q+1
                chunk = ch0 if bi == 0 else C[:, kA, :]
                sig = SIG[kind]
                poff = POFF[kind]
                out_tile = outp.tile([128, 8, H], bf16, name="ot")
                for q in range(4):
                    route = next_route()
                    ident = route == "IA"
                    ps = psum.tile([128, 2 * H], f32, tag="ps", name=f"ps{q}")
                    for s in range(2):
                        if kind == "F":
                            nc.tensor.matmul(
                                ps[:, s * H : (s + 1) * H],
                                ones_sb[32 * q : 32 * q + 1, :],
                                chunk[32 * q : 32 * q + 1, s * H : (s + 1) * H],
                                start=True,
                                stop=not ident,
                                tile_position=(32 * q, 0),
                            )
                        else:
                            nc.tensor.matmul(
                                ps[:, s * H : (s + 1) * H],
                                perm_sb[
                                    32 * q : 32 * q + 2,
                                    MOFF[kind] : MOFF[kind] + 128,
                                ],
                                chunk[32 * q : 32 * q + 2, s * H : (s + 1) * H],
                                start=True,
                                stop=not ident,
                                tile_position=(32 * q, 0),
                            )
                    if ident:
                        for s in range(2):
                            nc.tensor.matmul(
                                ps[:, s * H : (s + 1) * H],
                                perm_sb[:, poff : poff + 128],
                                P_rep[:, s, :],
                                start=False,
                                stop=True,
                                tile_position=(0, 0),
                            )
                    ps_v = ps.rearrange("p (u h) -> p u h", u=2)
                    out_sl = out_tile[:, 2 * q : 2 * q + 2, :]
                    if route == "IA":
                        nc.scalar.activation(out_sl, ps_v, COPY)
                    elif route == "TV":
                        nc.vector.tensor_tensor(
                            out=out_sl, in0=sig[:, :, :], in1=ps_v, op=ADD
                        )
                    else:  # SV / SG
                        bc = bcast.tile([128, 2, H], bf16, name="bc", tag="bc")
                        nc.scalar.activation(bc, ps_v, COPY)
                        eng = nc.vector if route == "SV" else nc.gpsimd
                        eng.tensor_tensor(
                            out=out_sl, in0=sig[:, :, :], in1=bc, op=ADD
                        )
                # exact-height writes, one per seg
                if kind == "F":
                    segs = [(kA, 0, 128, 0)]
                elif kind == "P96":
                    segs = [(kA, 0, 96, 32), (kB, 96, 128, 0)]
                else:
                    segs = [(kA, 0, 64, 64), (kB, 64, 128, 0)]
                # split big segs per column-half so each DMA waits on only
                # two groups' evictions; small segs (<64 rows) go whole on
                # the scalar ring
                for blk, plo, phi, sh in segs:
                    plo_eff = max(plo, 8 * blk - sh)
                    r0 = plo_eff + sh
                    if phi - plo_eff < 64:
                        out_dma(
                            out_ext[r0:128, 8 * blk : 8 * blk + 8, :],
                            out_tile[plo_eff:phi, :, :],
                            small=True,
                        )
                    else:
                        for ch_ in range(2):
                            out_dma(
                                out_ext[
                                    r0:128,
                                    8 * blk + 4 * ch_ : 8 * blk + 4 * ch_ + 4,
                                    :,
                                ],
                                out_tile[plo_eff:phi, 4 * ch_ : 4 * ch_ + 4, :],
                            )

            for bi, (kind, kA, kB) in enumerate(BTILES):
                do_btile(bi, kind, kA, kB)
    nc.compile()
    return nc


def _get_nc():
    if "nc" not in _BUILT:
        _BUILT["nc"] = _build_nc()
    return _BUILT["nc"]


def _make_perm():
    perm = np.zeros((128, 640), dtype=np.float32)
    p = np.arange(128)
    perm[p, p] = 1.0
    s96 = np.where(p < 96, p + 32, p)
    perm[s96, 128 + p] = 1.0
    s64 = np.where(p < 64, p + 64, p)
    perm[s64, 256 + p] = 1.0
    for q in range(4):
        perm[32 * q + 0, 384 + p] = (p < 96).astype(np.float32)
        perm[32 * q + 1, 384 + p] = (p >= 96).astype(np.float32)
        perm[32 * q + 0, 512 + p] = (p < 64).astype(np.float32)
        perm[32 * q + 1, 512 + p] = (p >= 64).astype(np.float32)
    return perm


def _make_in_maps(local_feats, W, b):
    import ml_dtypes

    bf = ml_dtypes.bfloat16
    local_feats = np.asarray(local_feats, dtype=np.float32)
    W = np.asarray(W, dtype=np.float32)
    b = np.asarray(b, dtype=np.float32)
    hb = np.ascontiguousarray((0.5 * b).reshape(1, H)).astype(bf)
    perm = _make_perm().astype(bf)
    base = np.zeros((H, WXW), dtype=np.float32)
    base[:, N : N + H] = W.T
    base[0, N + H :] = 1.0
    in_maps = []
    for c in range(NCORES):
        wx = base.copy()
        wx[:, :N] = local_feats[c].T
        in_maps.append({"wx": wx.astype(bf), "halfb": hb, "perm": perm})
    return in_maps


def _collect(res):
    iu, ju = np.triu_indices(16, 1)
    full = np.empty((NCORES, N, N, H), dtype=np.float32)
    for c in range(NCORES):
        a = np.asarray(res.results[c]["out"]).astype(np.float32)
        v = a.reshape(16, 8, 16, 8, H)
        v[iu, :, ju] = v[ju, :, iu].swapaxes(1, 2)
        full[c] = a
    return full


def kernel(local_feats, W, b):
    from concourse.bass_utils import run_bass_kernel_spmd

    nc = _get_nc()
    in_maps = _make_in_maps(local_feats, W, b)
    res = run_bass_kernel_spmd(nc, in_maps, core_ids=list(range(NCORES)))
    return _collect(res)


def run_profiled(local_feats, W, b, **trace_kwargs):
    """Like kernel() but with neuron-profile tracing; returns (out, results)."""
    from concourse.bass_utils import run_bass_kernel_spmd

    nc = _get_nc()
    in_maps = _make_in_maps(local_feats, W, b)
    res = run_bass_kernel_spmd(
        nc, in_maps, core_ids=list(range(NCORES)), trace=True, **trace_kwargs
    )
    return _collect(res), res


# revision 36
# speedup vs baseline: 1.1276x; 1.1276x over previous
"""Trainium2 Bass kernel for nn_Attention_86199993631321.

Reference computation (B=8, N=128, H=512):
    pair[b,i,j,:] = x[b,i,:] + x[b,j,:]
    out = pair @ W.T + b                # [B, N, N, H]

Algebra: out[b,i,j,:] = P[b,i,:] + P[b,j,:] with P = x @ W.T + 0.5*b.
Sharding: data-parallel over batch (core b handles batch b).

v3 design notes (HW facts measured on this setup):
  - PE runs at a fixed 1.2 GHz (no HAM warm-up observed); at most ~2
    matmul streams overlap across row-groups.  FD<=512 per matmul (one
    f32 PSUM bank).
  - PSUM->SBUF eviction runs at 1x everywhere (fp32 source), ~1 elem/
    cycle/lane: ACT (1.2GHz) and DVE (0.96GHz) are the only PSUM
    readers; GPSIMD has no PSUM port.
  - DMA: full-128-partition sources stream at ~370 GB/s; partition
    subranges lose proportional bandwidth ([0,64) even SDMA engines,
    [64,128) odd).
  - Output is symmetric: only the block-lower-triangle (8-col blocks)
    is computed/written (8.7MB bf16/core); host mirrors + upcasts.
  - Column blocks pack into 128-partition tiles at 32-row granularity:
    blocks 0-3 full; (4,12),(5,13),(6,14),(7,15) as 96+32; (8,10),
    (9,11) as 64+64.  Each btile = 4 groups of [128, 2*512] f32 (2
    PSUM banks); psum pool bufs=4 decouples the pipeline.
  - Per group (quadrant q = its column pair): K=1 (full tiles) or K=2
    (packed pairs, host-built 0/1 partition masks select seg A/B rows)
    broadcast matmuls write P[j]; the i term P[sigma(p)] is added by
    route: TV (DVE TT from PSUM), SV (ACT copy + DVE bf16 TT), SG
    (ACT copy + GPSIMD TT), IA (PE sigma-permutation matmul
    accumulate + ACT copy).
"""

import sys

if "/opt/trn_rl_repo" not in sys.path:
    sys.path.insert(0, "/opt/trn_rl_repo")

import numpy as np

B, N, H = 8, 128, 512
NCORES = 8
KC = H // 128
WXW = N + H + 128  # x.T | W.T | ones col
# perm/mask input [128, 640] bf16:
#   0:128 identity, 128:256 sigma96, 256:384 sigma64,
#   384:512 pair masks type96 (row 32q: [p<96], row 32q+1: [p>=96]),
#   512:640 pair masks type64
# F/pair btiles interleaved to smooth per-engine load; ends on a
# full-width F block so the DMA drain tail runs at full rate
BTILES = [
    ("F", 0, None),
    ("P96", 4, 12),
    ("F", 1, None),
    ("P96", 5, 13),
    ("F", 2, None),
    ("P96", 6, 14),
    ("P64", 8, 10),
    ("P96", 7, 15),
    ("P64", 9, 11),
    ("F", 3, None),
]
# 40 groups (4 per btile, quadrant-ordered).  First pair btile is
# all-IA (PE is idle during the ramp and it needs no sigma buffer);
# the last btiles avoid slow SG chains so the DMA drain isn't gated.
ROUTES = [
    "TV", "SV", "SG", "TV",   # F0
    "IA", "IA", "IA", "IA",   # P96 (4,12)
    "TV", "SG", "SV", "TV",   # F1
    "IA", "SV", "TV", "SG",   # P96 (5,13)
    "TV", "IA", "SV", "TV",   # F2
    "SG", "TV", "IA", "SV",   # P96 (6,14)
    "TV", "SV", "TV", "IA",   # P64 (8,10)
    "SG", "TV", "IA", "SV",   # P96 (7,15)
    "TV", "IA", "SV", "TV",   # P64 (9,11)
    "SV", "TV", "IA", "TV",   # F3
]

_BUILT = {}


def _build_nc():
    import concourse.bass as bass
    import concourse.bacc as bacc
    import concourse.tile as tile
    from concourse import mybir

    f32 = mybir.dt.float32
    bf16 = mybir.dt.bfloat16
    ADD = mybir.AluOpType.add
    COPY = mybir.ActivationFunctionType.Copy

    nc = bacc.Bacc()
    wx_ext = nc.declare_dram_parameter("wx", [H, WXW], bf16, isOutput=False)
    hb_ext = nc.declare_dram_parameter("halfb", [1, H], bf16, isOutput=False)
    perm_ext = nc.declare_dram_parameter("perm", [128, 640], bf16, isOutput=False)
    out_ext = nc.declare_dram_parameter("out", [N, N, H], bf16, isOutput=True)

    group_idx = [0]
    dma_idx = [0]

    with tile.TileContext(nc) as tc:
        with (
            tc.tile_pool(name="const", bufs=1) as const,
            tc.tile_pool(name="stage", bufs=10) as stage,
            tc.tile_pool(name="bcast", bufs=4) as bcast,
            tc.tile_pool(name="outp", bufs=6) as outp,
            tc.tile_pool(name="psum", bufs=4, space="PSUM") as psum,
            tc.tile_pool(name="dscr", bufs=1, space="DRAM") as dscr,
        ):
            # ---- load packed inputs ----
            wx_sb = const.tile([128, KC, WXW], bf16)
            wx_v = wx_ext.rearrange("(c p) m -> p c m", p=128)
            for c in range(KC):
                eng = nc.sync if c % 2 == 0 else nc.scalar
                eng.dma_start(out=wx_sb[:, c, :], in_=wx_v[:, c, :])
            perm_sb = const.tile([128, 640], bf16)
            nc.sync.dma_start(out=perm_sb, in_=perm_ext[:, :])
            ones_sb = const.tile([128, 128], bf16)
            nc.vector.memset(ones_sb, 1.0)
            hb_sb = const.tile([1, H], bf16)
            nc.gpsimd.dma_start(out=hb_sb, in_=hb_ext[:, :])

            # ---- P = x @ W.T + 0.5*b -> PSUM [128, 512] ----
            ps_proj = psum.tile([128, 2 * H], f32, tag="ps")
            for c in range(KC):
                nc.tensor.matmul(
                    ps_proj[:, 0:H],
                    wx_sb[:, c, 0:N],
                    wx_sb[:, c, N : N + H],
                    start=(c == 0),
                    stop=False,
                )
            nc.tensor.matmul(
                ps_proj[:, 0:H],
                wx_sb[0:1, 0, N + H : N + H + 128],
                hb_sb,
                start=False,
                stop=True,
            )

            # ---- P replicated 2x along free dim (bf16) ----
            P_rep = const.tile([128, 2, H], bf16)
            nc.scalar.activation(P_rep[:, 0, :], ps_proj[:, 0:H], COPY)
            nc.vector.tensor_copy(P_rep[:, 1, :], P_rep[:, 0, :])

            # Prestage ALL btile chunks on the GPS queue in btile order;
            # sigma staging is injected after the first four btiles'
            # chunks so the ramp-critical chunk DMAs land first.
            P_g96 = const.tile([128, 2, H], bf16)
            P_g64 = const.tile([128, 2, H], bf16)
            chunks = []
            for bi, (kind, kA, kB) in enumerate(BTILES):
                ch = stage.tile([128, 2 * H], bf16, name=f"ch{bi}", tag="chunk")
                nc.gpsimd.dma_start(
                    out=ch[0:128:32, :], in_=P_rep[8 * kA : 8 * kA + 8, 0, :]
                )
                if kind != "F":
                    nc.gpsimd.dma_start(
                        out=ch[1:128:32, :],
                        in_=P_rep[8 * kB : 8 * kB + 8, 0, :],
                    )
                chunks.append(ch)
                if bi == 3:
                    # sigma-shifted copies for TT routes on packed tiles
                    nc.gpsimd.dma_start(
                        out=P_g96[0:96, :, :], in_=P_rep[32:128, :, :]
                    )
                    nc.gpsimd.dma_start(
                        out=P_g96[96:128, :, :], in_=P_rep[96:128, :, :]
                    )
                    nc.gpsimd.dma_start(
                        out=P_g64[0:64, :, :], in_=P_rep[64:128, :, :]
                    )
                    nc.gpsimd.dma_start(
                        out=P_g64[64:128, :, :], in_=P_rep[64:128, :, :]
                    )
            SIG = {"F": P_rep, "P96": P_g96, "P64": P_g64}
            POFF = {"F": 0, "P96": 128, "P64": 256}
            MOFF = {"P96": 384, "P64": 512}

            def next_route():
                r = ROUTES[group_idx[0] % len(ROUTES)]
                group_idx[0] += 1
                return r

            def out_dma(dst, src, small=False):
                # large writes on the sync ring; small ones on the scalar
                # ring (cheap to issue) to keep the sync issue queue short
                dma_idx[0] += 1
                eng = nc.scalar if small else nc.sync
                eng.dma_start(out=dst, in_=src)

            def do_btile(bi, kind, kA, kB):
                # chunk: quadrant q partition 32q holds P rows (8kA+2q,
                # 8kA+2q+1); packed btiles add kB rows at partition 32q+1
                chunk = chunks[bi]
                sig = SIG[kind]
                poff = POFF[kind]
                out_tile = outp.tile([128, 8, H], bf16, name="ot")
                for q in range(4):
                    route = next_route()
                    ident = route == "IA"
                    ps = psum.tile([128, 2 * H], f32, tag="ps", name=f"ps{q}")
                    for s in range(2):
                        if kind == "F":
                            nc.tensor.matmul(
                                ps[:, s * H : (s + 1) * H],
                                ones_sb[32 * q : 32 * q + 1, :],
                                chunk[32 * q : 32 * q + 1, s * H : (s + 1) * H],
                                start=True,
                                stop=not ident,
                                tile_position=(32 * q, 0),
                            )
                        else:
                            nc.tensor.matmul(
                                ps[:, s * H : (s + 1) * H],
                                perm_sb[
                                    32 * q : 32 * q + 2,
                                    MOFF[kind] : MOFF[kind] + 128,
                                ],
                                chunk[32 * q : 32 * q + 2, s * H : (s + 1) * H],
                                start=True,
                                stop=not ident,
                                tile_position=(32 * q, 0),
                            )
                    if ident:
                        for s in range(2):
                            nc.tensor.matmul(
                                ps[:, s * H : (s + 1) * H],
                                perm_sb[:, poff : poff + 128],
                                P_rep[:, s, :],
                                start=False,
                                stop=True,
                                tile_position=(0, 0),
                            )
                    ps_v = ps.rearrange("p (u h) -> p u h", u=2)
                    out_sl = out_tile[:, 2 * q : 2 * q + 2, :]
                    if route == "IA":
                        nc.scalar.activation(out_sl, ps_v, COPY)
                    elif route == "TV":
                        nc.vector.tensor_tensor(
                            out=out_sl, in0=sig[:, :, :], in1=ps_v, op=ADD
                        )
                    else:  # SV / SG
                        bc = bcast.tile([128, 2, H], bf16, name="bc", tag="bc")
                        nc.scalar.activation(bc, ps_v, COPY)
                        eng = nc.vector if route == "SV" else nc.gpsimd
                        eng.tensor_tensor(
                            out=out_sl, in0=sig[:, :, :], in1=bc, op=ADD
                        )
                # exact-height writes, one per seg
                if kind == "F":
                    segs = [(kA, 0, 128, 0)]
                elif kind == "P96":
                    segs = [(kA, 0, 96, 32), (kB, 96, 128, 0)]
                else:
                    segs = [(kA, 0, 64, 64), (kB, 64, 128, 0)]
                # split big segs per column-half so each DMA waits on only
                # two groups' evictions; small segs (<64 rows) go whole on
                # the scalar ring
                for blk, plo, phi, sh in segs:
                    plo_eff = max(plo, 8 * blk - sh)
                    r0 = plo_eff + sh
                    if phi - plo_eff < 64:
                        out_dma(
                            out_ext[r0:128, 8 * blk : 8 * blk + 8, :],
                            out_tile[plo_eff:phi, :, :],
                            small=True,
                        )
                    else:
                        for ch_ in range(2):
                            out_dma(
                                out_ext[
                                    r0:128,
                                    8 * blk + 4 * ch_ : 8 * blk + 4 * ch_ + 4,
                                    :,
                                ],
                                out_tile[plo_eff:phi, 4 * ch_ : 4 * ch_ + 4, :],
                            )

            for bi, (kind, kA, kB) in enumerate(BTILES):
                do_btile(bi, kind, kA, kB)
    nc.compile()
    return nc


def _get_nc():
    if "nc" not in _BUILT:
        _BUILT["nc"] = _build_nc()
    return _BUILT["nc"]


def _make_perm():
    perm = np.zeros((128, 640), dtype=np.float32)
    p = np.arange(128)
    perm[p, p] = 1.0
    s96 = np.where(p < 96, p + 32, p)
    perm[s96, 128 + p] = 1.0
    s64 = np.where(p < 64, p + 64, p)
    perm[s64, 256 + p] = 1.0
    for q in range(4):
        perm[32 * q + 0, 384 + p] = (p < 96).astype(np.float32)
        perm[32 * q + 1, 384 + p] = (p >= 96).astype(np.float32)
        perm[32 * q + 0, 512 + p] = (p < 64).astype(np.float32)
        perm[32 * q + 1, 512 + p] = (p >= 64).astype(np.float32)
    return perm


def _make_in_maps(local_feats, W, b):
    import ml_dtypes

    bf = ml_dtypes.bfloat16
    local_feats = np.asarray(local_feats, dtype=np.float32)
    W = np.asarray(W, dtype=np.float32)
    b = np.asarray(b, dtype=np.float32)
    hb = np.ascontiguousarray((0.5 * b).reshape(1, H)).astype(bf)
    perm = _make_perm().astype(bf)
    base = np.zeros((H, WXW), dtype=np.float32)
    base[:, N : N + H] = W.T
    base[0, N + H :] = 1.0
    in_maps = []
    for c in range(NCORES):
        wx = base.copy()
        wx[:, :N] = local_feats[c].T
        in_maps.append({"wx": wx.astype(bf), "halfb": hb, "perm": perm})
    return in_maps


def _collect(res):
    iu, ju = np.triu_indices(16, 1)
    full = np.empty((NCORES, N, N, H), dtype=np.float32)
    for c in range(NCORES):
        a = np.asarray(res.results[c]["out"]).astype(np.float32)
        v = a.reshape(16, 8, 16, 8, H)
        v[iu, :, ju] = v[ju, :, iu].swapaxes(1, 2)
        full[c] = a
    return full


def kernel(local_feats, W, b):
    from concourse.bass_utils import run_bass_kernel_spmd

    nc = _get_nc()
    in_maps = _make_in_maps(local_feats, W, b)
    res = run_bass_kernel_spmd(nc, in_maps, core_ids=list(range(NCORES)))
    return _collect(res)


def run_profiled(local_feats, W, b, **trace_kwargs):
    """Like kernel() but with neuron-profile tracing; returns (out, results)."""
    from concourse.bass_utils import run_bass_kernel_spmd

    nc = _get_nc()
    in_maps = _make_in_maps(local_feats, W, b)
    res = run_bass_kernel_spmd(
        nc, in_maps, core_ids=list(range(NCORES)), trace=True, **trace_kwargs
    )
    return _collect(res), res


# revision 38
# speedup vs baseline: 1.1490x; 1.0190x over previous
"""Trainium2 Bass kernel for nn_Attention_86199993631321.

Reference computation (B=8, N=128, H=512):
    pair[b,i,j,:] = x[b,i,:] + x[b,j,:]
    out = pair @ W.T + b                # [B, N, N, H]

Algebra: out[b,i,j,:] = P[b,i,:] + P[b,j,:] with P = x @ W.T + 0.5*b.
Sharding: data-parallel over batch (core b handles batch b).

v3 design notes (HW facts measured on this setup):
  - PE runs at a fixed 1.2 GHz (no HAM warm-up observed); at most ~2
    matmul streams overlap across row-groups.  FD<=512 per matmul (one
    f32 PSUM bank).
  - PSUM->SBUF eviction runs at 1x everywhere (fp32 source), ~1 elem/
    cycle/lane: ACT (1.2GHz) and DVE (0.96GHz) are the only PSUM
    readers; GPSIMD has no PSUM port.
  - DMA: full-128-partition sources stream at ~370 GB/s; partition
    subranges lose proportional bandwidth ([0,64) even SDMA engines,
    [64,128) odd).
  - Output is symmetric: only the block-lower-triangle (8-col blocks)
    is computed/written (8.7MB bf16/core); host mirrors + upcasts.
  - Column blocks pack into 128-partition tiles at 32-row granularity:
    blocks 0-3 full; (4,12),(5,13),(6,14),(7,15) as 96+32; (8,10),
    (9,11) as 64+64.  Each btile = 4 groups of [128, 2*512] f32 (2
    PSUM banks); psum pool bufs=4 decouples the pipeline.
  - Per group (quadrant q = its column pair): K=1 (full tiles) or K=2
    (packed pairs, host-built 0/1 partition masks select seg A/B rows)
    broadcast matmuls write P[j]; the i term P[sigma(p)] is added by
    route: TV (DVE TT from PSUM), SV (ACT copy + DVE bf16 TT), SG
    (ACT copy + GPSIMD TT), IA (PE sigma-permutation matmul
    accumulate + ACT copy).
"""

import sys

if "/opt/trn_rl_repo" not in sys.path:
    sys.path.insert(0, "/opt/trn_rl_repo")

import numpy as np

B, N, H = 8, 128, 512
NCORES = 8
KC = H // 128
WXW = N + H + 128  # x.T | W.T | ones col
# perm/mask input [128, 640] bf16:
#   0:128 identity, 128:256 sigma96, 256:384 sigma64,
#   384:512 pair masks type96 (row 32q: [p<96], row 32q+1: [p>=96]),
#   512:640 pair masks type64
# F/pair btiles interleaved to smooth per-engine load; ends on a
# full-width F block so the DMA drain tail runs at full rate
BTILES = [
    ("F", 0, None),
    ("P96", 4, 12),
    ("F", 1, None),
    ("P96", 5, 13),
    ("F", 2, None),
    ("P96", 6, 14),
    ("P64", 8, 10),
    ("P96", 7, 15),
    ("P64", 9, 11),
    ("F", 3, None),
]
# 40 groups (4 per btile, quadrant-ordered).  First pair btile is
# all-IA (PE is idle during the ramp and it needs no sigma buffer);
# the last btiles avoid slow SG chains so the DMA drain isn't gated.
ROUTES = [
    "TV", "SV", "SG", "TV",   # F0
    "IA", "IA", "IA", "IA",   # P96 (4,12)
    "TV", "SG", "SV", "TV",   # F1
    "TV", "SV", "TV", "SG",   # P96 (5,13)
    "TV", "IA", "SV", "TV",   # F2
    "SG", "TV", "TV", "SV",   # P96 (6,14)
    "TV", "SV", "TV", "IA",   # P64 (8,10)
    "SG", "TV", "IA", "SV",   # P96 (7,15)
    "TV", "TV", "SV", "TV",   # P64 (9,11)
    "SV", "TV", "TV", "TV",   # F3
]

_BUILT = {}


def _build_nc():
    import concourse.bass as bass
    import concourse.bacc as bacc
    import concourse.tile as tile
    from concourse import mybir

    f32 = mybir.dt.float32
    bf16 = mybir.dt.bfloat16
    ADD = mybir.AluOpType.add
    COPY = mybir.ActivationFunctionType.Copy

    nc = bacc.Bacc()
    wx_ext = nc.declare_dram_parameter("wx", [H, WXW], bf16, isOutput=False)
    hb_ext = nc.declare_dram_parameter("halfb", [1, H], bf16, isOutput=False)
    perm_ext = nc.declare_dram_parameter("perm", [128, 640], bf16, isOutput=False)
    out_ext = nc.declare_dram_parameter("out", [N, N, H], bf16, isOutput=True)

    group_idx = [0]
    dma_idx = [0]

    with tile.TileContext(nc) as tc:
        with (
            tc.tile_pool(name="const", bufs=1) as const,
            tc.tile_pool(name="stage", bufs=10) as stage,
            tc.tile_pool(name="bcast", bufs=6) as bcast,
            tc.tile_pool(name="outp", bufs=6) as outp,
            tc.tile_pool(name="psum", bufs=4, space="PSUM") as psum,
            tc.tile_pool(name="dscr", bufs=1, space="DRAM") as dscr,
        ):
            # ---- load packed inputs ----
            wx_sb = const.tile([128, KC, WXW], bf16)
            wx_v = wx_ext.rearrange("(c p) m -> p c m", p=128)
            for c in range(KC):
                eng = nc.sync if c % 2 == 0 else nc.scalar
                eng.dma_start(out=wx_sb[:, c, :], in_=wx_v[:, c, :])
            perm_sb = const.tile([128, 640], bf16)
            nc.sync.dma_start(out=perm_sb, in_=perm_ext[:, :])
            ones_sb = const.tile([128, 128], bf16)
            nc.vector.memset(ones_sb, 1.0)
            hb_sb = const.tile([1, H], bf16)
            nc.gpsimd.dma_start(out=hb_sb, in_=hb_ext[:, :])

            # ---- P = x @ W.T + 0.5*b -> PSUM [128, 512] ----
            ps_proj = psum.tile([128, 2 * H], f32, tag="ps")
            for c in range(KC):
                nc.tensor.matmul(
                    ps_proj[:, 0:H],
                    wx_sb[:, c, 0:N],
                    wx_sb[:, c, N : N + H],
                    start=(c == 0),
                    stop=False,
                )
            nc.tensor.matmul(
                ps_proj[:, 0:H],
                wx_sb[0:1, 0, N + H : N + H + 128],
                hb_sb,
                start=False,
                stop=True,
            )

            # ---- P replicated 2x along free dim (bf16) ----
            P_rep = const.tile([128, 2, H], bf16)
            nc.scalar.activation(P_rep[:, 0, :], ps_proj[:, 0:H], COPY)
            nc.vector.tensor_copy(P_rep[:, 1, :], P_rep[:, 0, :])

            # Prestage ALL btile chunks on the GPS queue in btile order;
            # sigma staging is injected after the first four btiles'
            # chunks so the ramp-critical chunk DMAs land first.
            P_g96 = const.tile([128, 2, H], bf16)
            P_g64 = const.tile([128, 2, H], bf16)
            chunks = []
            for bi, (kind, kA, kB) in enumerate(BTILES):
                ch = stage.tile([128, 2 * H], bf16, name=f"ch{bi}", tag="chunk")
                nc.gpsimd.dma_start(
                    out=ch[0:128:32, :], in_=P_rep[8 * kA : 8 * kA + 8, 0, :]
                )
                if kind != "F":
                    nc.gpsimd.dma_start(
                        out=ch[1:128:32, :],
                        in_=P_rep[8 * kB : 8 * kB + 8, 0, :],
                    )
                chunks.append(ch)
                if bi == 3:
                    # sigma-shifted copies for TT routes on packed tiles
                    nc.gpsimd.dma_start(
                        out=P_g96[0:96, :, :], in_=P_rep[32:128, :, :]
                    )
                    nc.gpsimd.dma_start(
                        out=P_g96[96:128, :, :], in_=P_rep[96:128, :, :]
                    )
                    nc.gpsimd.dma_start(
                        out=P_g64[0:64, :, :], in_=P_rep[64:128, :, :]
                    )
                    nc.gpsimd.dma_start(
                        out=P_g64[64:128, :, :], in_=P_rep[64:128, :, :]
                    )
            SIG = {"F": P_rep, "P96": P_g96, "P64": P_g64}
            POFF = {"F": 0, "P96": 128, "P64": 256}
            MOFF = {"P96": 384, "P64": 512}

            def next_route():
                r = ROUTES[group_idx[0] % len(ROUTES)]
                group_idx[0] += 1
                return r

            def out_dma(dst, src, small=False):
                # large writes on the sync ring; small ones on the scalar
                # ring (cheap to issue) to keep the sync issue queue short
                dma_idx[0] += 1
                eng = nc.scalar if small else nc.sync
                eng.dma_start(out=dst, in_=src)

            def do_btile(bi, kind, kA, kB):
                # chunk: quadrant q partition 32q holds P rows (8kA+2q,
                # 8kA+2q+1); packed btiles add kB rows at partition 32q+1
                chunk = chunks[bi]
                sig = SIG[kind]
                poff = POFF[kind]
                out_tile = outp.tile([128, 8, H], bf16, name="ot")
                for q in range(4):
                    route = next_route()
                    ident = route == "IA"
                    ps = psum.tile([128, 2 * H], f32, tag="ps", name=f"ps{q}")
                    for s in range(2):
                        if kind == "F":
                            nc.tensor.matmul(
                                ps[:, s * H : (s + 1) * H],
                                ones_sb[32 * q : 32 * q + 1, :],
                                chunk[32 * q : 32 * q + 1, s * H : (s + 1) * H],
                                start=True,
                                stop=not ident,
                                tile_position=(32 * q, 0),
                            )
                        else:
                            nc.tensor.matmul(
                                ps[:, s * H : (s + 1) * H],
                                perm_sb[
                                    32 * q : 32 * q + 2,
                                    MOFF[kind] : MOFF[kind] + 128,
                                ],
                                chunk[32 * q : 32 * q + 2, s * H : (s + 1) * H],
                                start=True,
                                stop=not ident,
                                tile_position=(32 * q, 0),
                            )
                    if ident:
                        for s in range(2):
                            nc.tensor.matmul(
                                ps[:, s * H : (s + 1) * H],
                                perm_sb[:, poff : poff + 128],
                                P_rep[:, s, :],
                                start=False,
                                stop=True,
                                tile_position=(0, 0),
                            )
                    ps_v = ps.rearrange("p (u h) -> p u h", u=2)
                    out_sl = out_tile[:, 2 * q : 2 * q + 2, :]
                    if route == "IA":
                        nc.scalar.activation(out_sl, ps_v, COPY)
                    elif route == "TV":
                        nc.vector.tensor_tensor(
                            out=out_sl, in0=sig[:, :, :], in1=ps_v, op=ADD
                        )
                    else:  # SV / SG
                        bc = bcast.tile([128, 2, H], bf16, name="bc", tag="bc")
                        nc.scalar.activation(bc, ps_v, COPY)
                        eng = nc.vector if route == "SV" else nc.gpsimd
                        eng.tensor_tensor(
                            out=out_sl, in0=sig[:, :, :], in1=bc, op=ADD
                        )
                # exact-height writes, one per seg
                if kind == "F":
                    segs = [(kA, 0, 128, 0)]
                elif kind == "P96":
                    segs = [(kA, 0, 96, 32), (kB, 96, 128, 0)]
                else:
                    segs = [(kA, 0, 64, 64), (kB, 64, 128, 0)]
                # split big segs per column-half so each DMA waits on only
                # two groups' evictions; small segs (<64 rows) go whole on
                # the scalar ring
                for blk, plo, phi, sh in segs:
                    plo_eff = max(plo, 8 * blk - sh)
                    r0 = plo_eff + sh
                    if phi - plo_eff < 64:
                        out_dma(
                            out_ext[r0:128, 8 * blk : 8 * blk + 8, :],
                            out_tile[plo_eff:phi, :, :],
                            small=True,
                        )
                    else:
                        for ch_ in range(2):
                            out_dma(
                                out_ext[
                                    r0:128,
                                    8 * blk + 4 * ch_ : 8 * blk + 4 * ch_ + 4,
                                    :,
                                ],
                                out_tile[plo_eff:phi, 4 * ch_ : 4 * ch_ + 4, :],
                            )

            for bi, (kind, kA, kB) in enumerate(BTILES):
                do_btile(bi, kind, kA, kB)
    nc.compile()
    return nc


def _get_nc():
    if "nc" not in _BUILT:
        _BUILT["nc"] = _build_nc()
    return _BUILT["nc"]


def _make_perm():
    perm = np.zeros((128, 640), dtype=np.float32)
    p = np.arange(128)
    perm[p, p] = 1.0
    s96 = np.where(p < 96, p + 32, p)
    perm[s96, 128 + p] = 1.0
    s64 = np.where(p < 64, p + 64, p)
    perm[s64, 256 + p] = 1.0
    for q in range(4):
        perm[32 * q + 0, 384 + p] = (p < 96).astype(np.float32)
        perm[32 * q + 1, 384 + p] = (p >= 96).astype(np.float32)
        perm[32 * q + 0, 512 + p] = (p < 64).astype(np.float32)
        perm[32 * q + 1, 512 + p] = (p >= 64).astype(np.float32)
    return perm


def _make_in_maps(local_feats, W, b):
    import ml_dtypes

    bf = ml_dtypes.bfloat16
    local_feats = np.asarray(local_feats, dtype=np.float32)
    W = np.asarray(W, dtype=np.float32)
    b = np.asarray(b, dtype=np.float32)
    hb = np.ascontiguousarray((0.5 * b).reshape(1, H)).astype(bf)
    perm = _make_perm().astype(bf)
    base = np.zeros((H, WXW), dtype=np.float32)
    base[:, N : N + H] = W.T
    base[0, N + H :] = 1.0
    in_maps = []
    for c in range(NCORES):
        wx = base.copy()
        wx[:, :N] = local_feats[c].T
        in_maps.append({"wx": wx.astype(bf), "halfb": hb, "perm": perm})
    return in_maps


def _collect(res):
    iu, ju = np.triu_indices(16, 1)
    full = np.empty((NCORES, N, N, H), dtype=np.float32)
    for c in range(NCORES):
        a = np.asarray(res.results[c]["out"]).astype(np.float32)
        v = a.reshape(16, 8, 16, 8, H)
        v[iu, :, ju] = v[ju, :, iu].swapaxes(1, 2)
        full[c] = a
    return full


def kernel(local_feats, W, b):
    from concourse.bass_utils import run_bass_kernel_spmd

    nc = _get_nc()
    in_maps = _make_in_maps(local_feats, W, b)
    res = run_bass_kernel_spmd(nc, in_maps, core_ids=list(range(NCORES)))
    return _collect(res)


def run_profiled(local_feats, W, b, **trace_kwargs):
    """Like kernel() but with neuron-profile tracing; returns (out, results)."""
    from concourse.bass_utils import run_bass_kernel_spmd

    nc = _get_nc()
    in_maps = _make_in_maps(local_feats, W, b)
    res = run_bass_kernel_spmd(
        nc, in_maps, core_ids=list(range(NCORES)), trace=True, **trace_kwargs
    )
    return _collect(res), res
